# revision 26
# baseline (speedup 1.0000x reference)
"""Distributed ISTFT kernel for Trainium2 (8 NeuronCores, Bass/Tile).

Math (matches the jax reference):
  z: [2, 513, T] one-sided spectrum (real/imag), T = 8192 frames.
  Hermitian extension + ifft(1024) + window + overlap-add (hop 256) +
  divide by overlapped window sum + trim 512 each side -> [2, 2096896].

Structure (even/odd spectral symmetry):
  * real(ifft) frame x_f[n] = sum_kappa A[kappa, n] X[kappa, f] where A
    packs cos rows (zr bins 0..512) and sin rows (zi bins 1..511).
    Since A[k, n+512] = (-1)^k A[k, n], split rows into even-k / odd-k
    halves and compute only n = 0..511:
        E_f = We^T Xe,  O_f = Wo^T Xo          (half-width matmuls)
        x_f[n] = E+O,  x_f[n+512] = E-O        (DVE combine)
    The 1/win_sum normalization is 256-periodic in n, identical for the
    n and n+512 uses, so it folds into We/Wo columns on the host.  The
    window itself does not (w[n] != w[n+512]); it is applied by one
    broadcast multiply per half (Uw = w[0:512] (E+O), Vw = w[512:] (E-O)).
  * Overlap-add: out block b, sample r:
      O[b,r] = Uw[b+3, r] + Uw[b+2, 256+r] + Vw[b+1, r] + Vw[b, 256+r]
    The +1/+2/+3 frame-partition shifts are illegal for engine APs, so
    they run as shifted-identity matmuls accumulating in PSUM (the
    shifted diagonals are two host [128, 131] strips sliced per shift).
  * imag(ifft) is rank-2 -> one K=8 matmul per output tile with
    host-preshifted tap rows (unchanged from the direct version).
  * All matmul operands bf16; outputs written bf16, host upcasts.
  * Frame axis sharded 1024 output blocks/core with a 3-frame halo; no
    cross-core communication.  Window-sum edge fixup (global blocks 0
    and 8190, 512 samples) is applied host-side after the gather.
"""

import numpy as np

N_FFT = 1024
HOP = 256
T_FRAMES = 8192
N_CORES = 8
F_SLOTS = 1027   # valid frame slots per core (1024 owned blocks + 3 halo)
F2 = 1152        # padded slots = 9 tiles of 128
NB = 1024        # output blocks computed per core (core 7 uses 1023)

_CACHE = {}


def _build_nc():
    from contextlib import ExitStack

    import concourse.tile as tile
    from concourse import bacc, mybir

    f32 = mybir.dt.float32
    bf16 = mybir.dt.bfloat16

    nc = bacc.Bacc("TRN2", target_bir_lowering=False, debug=False,
                   num_devices=N_CORES)

    x_d = nc.dram_tensor("x", [1024, F2], bf16, kind="ExternalInput")
    w_d = nc.dram_tensor("wmat", [1024, 512], bf16, kind="ExternalInput")
    ones_d = nc.dram_tensor("onesr", [1, 128], bf16, kind="ExternalInput")
    wrow_d = nc.dram_tensor("wrows", [2, 512], bf16, kind="ExternalInput")
    id_d = nc.dram_tensor("idmats", [128, 896], bf16, kind="ExternalInput")
    t_d = nc.dram_tensor("taps", [8, NB], bf16, kind="ExternalInput")
    tw_d = nc.dram_tensor("tapw", [8, 256], bf16, kind="ExternalInput")
    o_d = nc.dram_tensor("out", [2, NB, 256], bf16, kind="ExternalOutput")

    with tile.TileContext(nc) as tc, ExitStack() as ctx:
        big = ctx.enter_context(tc.tile_pool(name="big", bufs=1))
        sml = ctx.enter_context(tc.tile_pool(name="sml", bufs=1))
        esb = ctx.enter_context(tc.tile_pool(name="esb", bufs=1))
        uvp = ctx.enter_context(tc.tile_pool(name="uvp", bufs=1))
        utm = ctx.enter_context(tc.tile_pool(name="utm", bufs=4))
        osb = ctx.enter_context(tc.tile_pool(name="osb", bufs=8))

        SY, SC, GP = nc.sync, nc.scalar, nc.gpsimd

        # ---- tiny setup loads + window broadcast tiles (transient psum)
        onesr = sml.tile([1, 128], bf16, tag="onesr")
        GP.dma_start(out=onesr[:], in_=ones_d.ap())
        wrowU = sml.tile([1, 512], bf16, tag="wrowU")
        GP.dma_start(out=wrowU[:], in_=wrow_d.ap()[0:1, :])
        wrowV = sml.tile([1, 512], bf16, tag="wrowV")
        GP.dma_start(out=wrowV[:], in_=wrow_d.ap()[1:2, :])
        idm = sml.tile([128, 896], bf16, tag="idm")
        GP.dma_start(out=idm[:], in_=id_d.ap())

        # idm packs 7 [128,128] matrices: shifts M_s (s=0..3) at cols
        # 128*s (M_s[k, m] = 1 iff k == m+s -> out[m] = rhs[m+s]), spills
        # Sp_s (s=1..3) at cols 128*(3+s) (1 iff k == m-(128-s)).
        def idmat(i):
            return idm[:, 128 * i:128 * (i + 1)]

        wUb = sml.tile([128, 512], f32, tag="wUb")
        wVb = sml.tile([128, 512], f32, tag="wVb")
        with tc.tile_pool(name="psw", bufs=1, space="PSUM") as psw:
            ps_wb = psw.tile([128, 512], f32, tag="ps_wb")
            nc.tensor.matmul(ps_wb[:], lhsT=onesr[:], rhs=wrowU[:],
                             start=True, stop=True)
            nc.scalar.copy(wUb[:], ps_wb[:])
            ps_wb2 = psw.tile([128, 512], f32, tag="ps_wb")
            nc.tensor.matmul(ps_wb2[:], lhsT=onesr[:], rhs=wrowV[:],
                             start=True, stop=True)
            nc.scalar.copy(wVb[:], ps_wb2[:])

        mainps = ctx.enter_context(tc.tile_pool(name="mainps", bufs=6,
                                                space="PSUM"))
        psqr = ctx.enter_context(tc.tile_pool(name="psqr", bufs=2,
                                              space="PSUM"))

        # ---- big input chunks interleaved across the 3 DMA queues in
        # k-order: pair (xs[k], W[k]) lands just ahead of the k-sweeps.
        xs = [big.tile([128, F2], bf16, tag=f"xs{k}", name=f"xs{k}")
              for k in range(8)]
        ws = [big.tile([128, 512], bf16, tag=f"ws{k}", name=f"ws{k}")
              for k in range(8)]
        qrot = [SY, SC, GP]
        qi = 0
        for k in range(8):
            qrot[qi % 3].dma_start(out=xs[k][:],
                                   in_=x_d.ap()[128 * k:128 * (k + 1), :])
            qi += 1
            qrot[qi % 3].dma_start(out=ws[k][:],
                                   in_=w_d.ap()[128 * k:128 * (k + 1), :])
            qi += 1

        taps = sml.tile([8, NB], bf16, tag="taps")
        GP.dma_start(out=taps[:], in_=t_d.ap())
        tapw = sml.tile([8, 256], bf16, tag="tapw")
        GP.dma_start(out=tapw[:], in_=tw_d.ap())

        # ---- main E/O sweeps --------------------------------------------
        E_sb = [esb.tile([128, 512], f32, tag=f"E{t}", name=f"E{t}")
                for t in range(9)]
        Uw = [uvp.tile([128, 512], bf16, tag=f"U{t}", name=f"U{t}")
              for t in range(9)]
        Vw = [uvp.tile([128, 512], bf16, tag=f"V{t}", name=f"V{t}")
              for t in range(9)]

        def prep(t, psO):
            # U = E + O (DVE), Uw = U * w[0:512] (Pool)
            # V = E - O (DVE), Vw = V * w[512:]  (DVE)
            u = utm.tile([128, 512], f32, tag="uraw", name=f"uraw{t}")
            nc.vector.tensor_add(u[:], E_sb[t][:], psO[:])
            nc.gpsimd.tensor_mul(Uw[t][:], u[:], wUb[:])
            v = utm.tile([128, 512], f32, tag="vraw", name=f"vraw{t}")
            nc.vector.tensor_sub(v[:], E_sb[t][:], psO[:])
            nc.vector.tensor_mul(Vw[t][:], v[:], wVb[:])

        # G-A: frame tiles 0..5, k-outer (streams with the DMA feed)
        psE = {t: mainps.tile([128, 512], f32, tag="mps", name=f"psE{t}")
               for t in range(6)}
        for k in range(4):
            for t in range(6):
                nc.tensor.matmul(psE[t][:],
                                 lhsT=xs[k][:, 128 * t:128 * t + 128],
                                 rhs=ws[k][:], start=(k == 0), stop=(k == 3))
        for t in range(6):
            nc.scalar.copy(E_sb[t][:], psE[t][:])
        psO = {t: mainps.tile([128, 512], f32, tag="mps", name=f"psO{t}")
               for t in range(6)}
        for k in range(4, 8):
            for t in range(6):
                nc.tensor.matmul(psO[t][:],
                                 lhsT=xs[k][:, 128 * t:128 * t + 128],
                                 rhs=ws[k][:], start=(k == 4), stop=(k == 7))
        for t in range(6):
            prep(t, psO[t])

        # G-B: frame tiles 6..8, k-inner (chunks all resident by now)
        for t in range(6, 9):
            pe = mainps.tile([128, 512], f32, tag="mps", name=f"psE{t}")
            for k in range(4):
                nc.tensor.matmul(pe[:], lhsT=xs[k][:, 128 * t:128 * t + 128],
                                 rhs=ws[k][:], start=(k == 0), stop=(k == 3))
            nc.scalar.copy(E_sb[t][:], pe[:])
            po = mainps.tile([128, 512], f32, tag="mps", name=f"psO{t}")
            for k in range(4, 8):
                nc.tensor.matmul(po[:], lhsT=xs[k][:, 128 * t:128 * t + 128],
                                 rhs=ws[k][:], start=(k == 4), stop=(k == 7))
            prep(t, po)

        # ---- combine + ch1 ----------------------------------------------
        oq = [SY, GP, SC]
        ev_state = {"n": 0}

        def evict(ps, tt, ch):
            i = ev_state["n"]
            ev_state["n"] += 1
            o = osb.tile([128, 256], bf16, tag=f"o{ch}", name=f"o{ch}_{tt}")
            if i % 2 == 0:
                nc.vector.tensor_copy(o[:], ps[:])
            else:
                nc.scalar.copy(o[:], ps[:])
            oq[i % 3].dma_start(
                out=o_d.ap()[ch:ch + 1, tt * 128:(tt + 1) * 128, :], in_=o[:])

        def combine(tt):
            ps = psqr.tile([128, 256], f32, tag="psq", name=f"psq{tt}")
            # T0: Uw[tt][p+3, 0:256] (+ spill from tile tt+1)
            nc.tensor.matmul(ps[:], lhsT=idmat(3), rhs=Uw[tt][:, 0:256],
                             start=True, stop=False)
            nc.tensor.matmul(ps[:], lhsT=idmat(3 + 3),
                             rhs=Uw[tt + 1][:, 0:256], start=False, stop=False)
            # T1: Uw[tt][p+2, 256:512]
            nc.tensor.matmul(ps[:], lhsT=idmat(2), rhs=Uw[tt][:, 256:512],
                             start=False, stop=False)
            nc.tensor.matmul(ps[:], lhsT=idmat(3 + 2),
                             rhs=Uw[tt + 1][:, 256:512], start=False, stop=False)
            # T2: Vw[tt][p+1, 0:256]
            nc.tensor.matmul(ps[:], lhsT=idmat(1), rhs=Vw[tt][:, 0:256],
                             start=False, stop=False)
            nc.tensor.matmul(ps[:], lhsT=idmat(3 + 1),
                             rhs=Vw[tt + 1][:, 0:256], start=False, stop=False)
            # T3: Vw[tt][p, 256:512] (aligned)
            nc.tensor.matmul(ps[:], lhsT=idmat(0), rhs=Vw[tt][:, 256:512],
                             start=False, stop=True)
            evict(ps, tt, 0)

        def ch1_group(tt):
            ps1 = psqr.tile([128, 256], f32, tag="psq", name=f"ps1_{tt}")
            nc.tensor.matmul(ps1[:], lhsT=taps[:, tt * 128:tt * 128 + 128],
                             rhs=tapw[:], start=True, stop=True)
            evict(ps1, tt, 1)

        for tt in range(8):
            combine(tt)
            ch1_group(tt)

    nc.compile()
    return nc


def _host_tensors(z: np.ndarray, window: np.ndarray):
    """Window-dependent folds, done once per call (host time is free)."""
    import ml_dtypes

    w = window.astype(np.float64)
    ws4 = w[0:256] + w[256:512] + w[512:768] + w[768:1024]
    n4 = np.where(ws4 >= 1e-6, 1.0 / np.where(ws4 >= 1e-6, ws4, 1.0), 1.0)
    ws3a = ws4 - w[768:1024]   # block 0 misses frame -1   (q=3 term)
    ws3b = ws4 - w[0:256]      # block 8190 misses frame 8192 (q=0 term)
    n3a = np.where(ws3a >= 1e-6, 1.0 / np.where(ws3a >= 1e-6, ws3a, 1.0), 1.0)
    n3b = np.where(ws3b >= 1e-6, 1.0 / np.where(ws3b >= 1e-6, ws3b, 1.0), 1.0)

    # A rows (cos 0..512, sin 1..511) over n = 0..511, with n4 folded in.
    j = np.arange(512, dtype=np.float64)[None, :]
    kc = np.arange(513, dtype=np.float64)[:, None]
    g = np.full((513, 1), 2.0)
    g[0, 0] = 1.0
    g[512, 0] = 1.0
    C = (g / N_FFT) * np.cos(2.0 * np.pi * kc * j / N_FFT)
    ks = np.arange(1, 512, dtype=np.float64)[:, None]
    S = (-2.0 / N_FFT) * np.sin(2.0 * np.pi * ks * j / N_FFT)
    n4f = np.tile(n4, 2)[None, :]
    C *= n4f
    S *= n4f
    # even/odd packing: We = [cos k even (257); sin k even (255)]
    #                   Wo = [cos k odd (256); sin k odd (256)]
    We = np.concatenate([C[0::2], S[1::2]], 0)          # [257+255=512, 512]
    Wo = np.concatenate([C[1::2], S[0::2]], 0)          # [256+256=512, 512]
    wmat = np.concatenate([We, Wo], 0).astype(ml_dtypes.bfloat16)

    # window rows for the U/V broadcast multiplies
    wrows = np.stack([w[0:512], w[512:1024]]).astype(ml_dtypes.bfloat16)

    # ch1 tap weights: rows 0-3 -> w[256q+r]*n4[r]/N ; rows 4-7 -> *(-1)^r
    w4 = w.reshape(4, 256)
    sgn = (1.0 - 2.0 * (np.arange(256) % 2))
    tapw = np.empty((8, 256), np.float64)
    tapw[0:4] = w4 * n4[None, :] / N_FFT
    tapw[4:8] = tapw[0:4] * sgn[None, :]
    tapw = tapw.astype(ml_dtypes.bfloat16)

    fx0 = (n3a * np.where(ws4 >= 1e-6, ws4, 1.0)).astype(np.float32)
    fx7 = (n3b * np.where(ws4 >= 1e-6, ws4, 1.0)).astype(np.float32)
    return wmat, wrows, tapw, fx0, fx7


def _id_strips():
    """7 stacked [128,128] matrices: shifts M_s (out[m]=rhs[m+s], s=0..3)
    then spills Sp_s (out[m]=rhs[m-(128-s)], s=1..3)."""
    import ml_dtypes
    idm = np.zeros((128, 896), np.float32)
    for s in range(4):
        for m in range(128 - s):
            idm[m + s, 128 * s + m] = 1.0          # M_s[k=m+s, m] = 1
    for i, s in enumerate((1, 2, 3)):
        for m in range(128 - s, 128):
            idm[m - (128 - s), 128 * (4 + i) + m] = 1.0
    return idm.astype(ml_dtypes.bfloat16)


def _inputs_for_cores(z: np.ndarray, window: np.ndarray):
    import ml_dtypes

    wmat, wrows, tapw, fx0, fx7 = _host_tensors(z, window)
    _CACHE["fx"] = (fx0, fx7)
    idm = _CACHE.get("idm")
    if idm is None:
        idm = _id_strips()
        _CACHE["idm"] = idm
    ones = np.ones((1, 128), ml_dtypes.bfloat16)

    # zero-padded zi0 / zi512 rows over all frame slots (halo = 3)
    zi0 = np.zeros(T_FRAMES + 6, np.float64)
    zi0[3:3 + T_FRAMES] = z[1, 0, :]
    zi512 = np.zeros(T_FRAMES + 6, np.float64)
    zi512[3:3 + T_FRAMES] = z[1, 512, :]

    in_maps = []
    for c in range(N_CORES):
        G = 1024 * c - 1  # global frame index of slot 0
        X = np.zeros((1024, F2), np.float32)
        lo, hi = max(0, G), min(T_FRAMES, G + F_SLOTS)
        s0, s1 = lo - G, hi - G
        # raw packing: rows 0..512 = zr0..512, rows 513..1023 = zi1..511
        zr = z[0, :, lo:hi]          # [513, n]
        zi = z[1, 1:512, lo:hi]      # [511, n]
        # even/odd packing to match We/Wo row order
        X[0:257, s0:s1] = zr[0::2]
        X[257:512, s0:s1] = zi[1::2]     # zi_k k even = rows 2,4,..,510
        X[512:768, s0:s1] = zr[1::2]
        X[768:1024, s0:s1] = zi[0::2]    # zi_k k odd = 1,3,..,511

        # taps[q, b] = zi0[G + b + 3 - q], taps[4+q, b] = zi512[...]
        taps = np.empty((8, NB), np.float64)
        for q in range(4):
            base = G + 3 - q + 3
            taps[q] = zi0[base:base + NB]
            taps[4 + q] = zi512[base:base + NB]

        in_maps.append({
            "x": X.astype(ml_dtypes.bfloat16),
            "wmat": wmat,
            "onesr": ones,
            "wrows": wrows,
            "idmats": idm,
            "taps": taps.astype(ml_dtypes.bfloat16),
            "tapw": tapw,
        })
    return in_maps


def kernel(z: np.ndarray, window: np.ndarray) -> np.ndarray:
    from concourse.bass_utils import run_bass_kernel_spmd

    z = np.asarray(z, dtype=np.float32)
    window = np.asarray(window, dtype=np.float32)

    nc = _CACHE.get("nc")
    if nc is None:
        nc = _build_nc()
        _CACHE["nc"] = nc

    in_maps = _inputs_for_cores(z, window)
    res = run_bass_kernel_spmd(nc, in_maps, list(range(N_CORES)))

    parts = []
    for c in range(N_CORES):
        nb = NB if c < N_CORES - 1 else NB - 1
        o = np.asarray(res.results[c]["out"], dtype=np.float32)  # [2, NB, 256]
        parts.append(o[:, :nb, :].reshape(2, -1))
    out = np.ascontiguousarray(np.concatenate(parts, axis=1))
    # edge-block window-sum fixup (blocks 0 and 8190), host-side
    fx0, fx7 = _CACHE["fx"]
    out[:, 0:256] *= fx0[None, :]
    out[:, -256:] *= fx7[None, :]
    return out


# revision 28
# speedup vs baseline: 1.2076x; 1.2076x over previous
"""Distributed ISTFT kernel for Trainium2 (8 NeuronCores, Bass/Tile).

Math (matches the jax reference):
  z: [2, 513, T] one-sided spectrum (real/imag), T = 8192 frames.
  Hermitian extension + ifft(1024) + window + overlap-add (hop 256) +
  divide by overlapped window sum + trim 512 each side -> [2, 2096896].

Structure (even/odd spectral symmetry, per-tile pipeline):
  * real(ifft) frame x_f[n] = sum_kappa A[kappa, n] X[kappa, f] where A
    packs cos rows (zr bins 0..512) and sin rows (zi bins 1..511).
    Since A[k, n+512] = (-1)^k A[k, n], split rows into even-k / odd-k
    halves and compute only n = 0..511 (half the matmul rows):
        E_f = We^T Xe,  O_f = Wo^T Xo
        x_f[n] = E+O,  x_f[n+512] = E-O
    The 1/win_sum normalization is 256-periodic in n, identical for the
    n and n+512 uses, so it folds into We/Wo columns on the host.  The
    window itself does not (w[n] != w[n+512]); it is applied with the
    +- combine as four bf16 DVE ops per tile (Uw = w[:512]*(E+O),
    Vw = w[512:]*(E-O)).
  * Frames are tiled 128 slots per tile at a 125 stride; output tiles
    are 125 blocks, so an output tile's whole frame range (125+3) lives
    in ONE frame tile.  Overlap-add for block b, sample r:
      O[b,r] = Uw[b+3, r] + Uw[b+2, 256+r] + Vw[b+1, r] + Vw[b, 256+r]
    The +1/+2/+3 partition shifts are illegal for engine APs, so they
    run as 4 shifted-identity matmuls accumulating in PSUM.
  * X is repacked tile-major on the host (one DMA per frame tile) so
    each tile's matmuls are independent of the rest of the stream: the
    kernel is a 9-deep per-tile pipeline and the PE never idles.
  * imag(ifft) is rank-2 -> one K=8 matmul per output tile with
    host-preshifted tap rows.
  * All matmul operands bf16; outputs written bf16, host upcasts.
  * Frame axis sharded 1024 output blocks/core with a 3-frame halo; no
    cross-core communication.  Window-sum edge fixup (global blocks 0
    and 8190, 512 samples) is applied host-side after the gather.
"""

import numpy as np

N_FFT = 1024
HOP = 256
T_FRAMES = 8192
N_CORES = 8
F_SLOTS = 1027   # valid frame slots per core (1024 owned blocks + 3 halo)
FPAD = 1128      # padded slots: tile 8 covers 1000..1127
NT = 9           # frame/output tiles per core
TS = 125         # output-tile stride (blocks per tile)
NB = 1024        # output blocks computed per core (core 7 uses 1023)

_CACHE = {}


def _build_nc():
    from contextlib import ExitStack

    import concourse.tile as tile
    from concourse import bacc, mybir

    f32 = mybir.dt.float32
    bf16 = mybir.dt.bfloat16

    nc = bacc.Bacc("TRN2", target_bir_lowering=False, debug=False,
                   num_devices=N_CORES)

    # tile-major X: rows 128*t..128*t+127 hold frame tile t as
    # [kappa-in-chunk, 128*k + slot]
    x_d = nc.dram_tensor("xt", [128 * NT, 1024], bf16, kind="ExternalInput")
    # chunk-major W: w8[p, 512*k + j] = W[128*k + p, j]
    w_d = nc.dram_tensor("w8", [128, 4096], bf16, kind="ExternalInput")
    ones_d = nc.dram_tensor("onesr", [1, 128], bf16, kind="ExternalInput")
    wrow_d = nc.dram_tensor("wrows", [2, 512], bf16, kind="ExternalInput")
    id_d = nc.dram_tensor("idmats", [128, 512], bf16, kind="ExternalInput")
    t_d = nc.dram_tensor("taps", [8, FPAD], bf16, kind="ExternalInput")
    tw_d = nc.dram_tensor("tapw", [8, 256], bf16, kind="ExternalInput")
    o_d = nc.dram_tensor("out", [2, NB, 256], bf16, kind="ExternalOutput")

    with tile.TileContext(nc) as tc, ExitStack() as ctx:
        big = ctx.enter_context(tc.tile_pool(name="big", bufs=1))
        xtp = ctx.enter_context(tc.tile_pool(name="xtp", bufs=3))
        sml = ctx.enter_context(tc.tile_pool(name="sml", bufs=1))
        eop = ctx.enter_context(tc.tile_pool(name="eop", bufs=3))
        uvp = ctx.enter_context(tc.tile_pool(name="uvp", bufs=3))
        osb = ctx.enter_context(tc.tile_pool(name="osb", bufs=6))

        SY, SC, GP = nc.sync, nc.scalar, nc.gpsimd

        # ---- tiny setup loads + window broadcast tiles (transient psum)
        onesr = sml.tile([1, 128], bf16, tag="onesr")
        GP.dma_start(out=onesr[:], in_=ones_d.ap())
        wrowU = sml.tile([1, 512], bf16, tag="wrowU")
        GP.dma_start(out=wrowU[:], in_=wrow_d.ap()[0:1, :])
        wrowV = sml.tile([1, 512], bf16, tag="wrowV")
        GP.dma_start(out=wrowV[:], in_=wrow_d.ap()[1:2, :])
        idm = sml.tile([128, 512], bf16, tag="idm")
        GP.dma_start(out=idm[:], in_=id_d.ap())

        wUb = sml.tile([128, 512], bf16, tag="wUb")
        wVb = sml.tile([128, 512], bf16, tag="wVb")
        with tc.tile_pool(name="psw", bufs=1, space="PSUM") as psw:
            ps_wb = psw.tile([128, 512], f32, tag="ps_wb")
            nc.tensor.matmul(ps_wb[:], lhsT=onesr[:], rhs=wrowU[:],
                             start=True, stop=True)
            nc.scalar.copy(wUb[:], ps_wb[:])
            ps_wb2 = psw.tile([128, 512], f32, tag="ps_wb")
            nc.tensor.matmul(ps_wb2[:], lhsT=onesr[:], rhs=wrowV[:],
                             start=True, stop=True)
            nc.scalar.copy(wVb[:], ps_wb2[:])

        mainps = ctx.enter_context(tc.tile_pool(name="mainps", bufs=4,
                                                space="PSUM"))
        psqr = ctx.enter_context(tc.tile_pool(name="psqr", bufs=3,
                                              space="PSUM"))

        # ---- weights first (3 pieces), then per-tile X DMAs
        ws = big.tile([128, 4096], bf16, tag="w8")
        SY.dma_start(out=ws[:, 0:1536], in_=w_d.ap()[:, 0:1536])
        SC.dma_start(out=ws[:, 1536:3072], in_=w_d.ap()[:, 1536:3072])
        GP.dma_start(out=ws[:, 3072:4096], in_=w_d.ap()[:, 3072:4096])

        qrot = [SY, SC, GP]
        xt = []
        for t in range(NT):
            x = xtp.tile([128, 1024], bf16, tag="xt", name=f"xt{t}")
            qrot[t % 3].dma_start(out=x[:],
                                  in_=x_d.ap()[128 * t:128 * (t + 1), :])
            xt.append(x)

        taps = sml.tile([8, FPAD], bf16, tag="taps")
        GP.dma_start(out=taps[:], in_=t_d.ap())
        tapw = sml.tile([8, 256], bf16, tag="tapw")
        GP.dma_start(out=tapw[:], in_=tw_d.ap())

        oq = [SY, GP, SC]
        ev_state = {"n": 0}

        # edge-block window-sum fixups (global blocks 0 / 8190) are
        # applied host-side after the gather.
        def evict(ps, tt, ch, nrows):
            i = ev_state["n"]
            ev_state["n"] += 1
            o = osb.tile([128, 256], bf16, tag=f"o{ch}", name=f"o{ch}_{tt}")
            nc.vector.tensor_copy(o[0:nrows, :], ps[0:nrows, :])
            oq[i % 3].dma_start(
                out=o_d.ap()[ch:ch + 1, TS * tt:TS * tt + nrows, :],
                in_=o[0:nrows, :])

        # ---- 9-deep per-tile pipeline ------------------------------------
        for t in range(NT):
            x = xt[t]
            psE = mainps.tile([128, 512], f32, tag="mps", name=f"psE{t}")
            for k in range(4):
                nc.tensor.matmul(psE[:], lhsT=x[:, 128 * k:128 * (k + 1)],
                                 rhs=ws[:, 512 * k:512 * (k + 1)],
                                 start=(k == 0), stop=(k == 3))
            eb = eop.tile([128, 512], bf16, tag="eb", name=f"eb{t}")
            nc.scalar.copy(eb[:], psE[:])
            psO = mainps.tile([128, 512], f32, tag="mps", name=f"psO{t}")
            for k in range(4, 8):
                nc.tensor.matmul(psO[:], lhsT=x[:, 128 * k:128 * (k + 1)],
                                 rhs=ws[:, 512 * k:512 * (k + 1)],
                                 start=(k == 4), stop=(k == 7))
            ob = eop.tile([128, 512], bf16, tag="ob", name=f"ob{t}")
            nc.scalar.copy(ob[:], psO[:])

            # bf16 SBUF prep on DVE (4x mode)
            u = uvp.tile([128, 512], bf16, tag="u", name=f"u{t}")
            nc.vector.tensor_add(u[:], eb[:], ob[:])
            uw = uvp.tile([128, 512], bf16, tag="uw", name=f"uw{t}")
            nc.vector.tensor_mul(uw[:], u[:], wUb[:])
            v = uvp.tile([128, 512], bf16, tag="v", name=f"v{t}")
            nc.vector.tensor_sub(v[:], eb[:], ob[:])
            vw = uvp.tile([128, 512], bf16, tag="vw", name=f"vw{t}")
            nc.vector.tensor_mul(vw[:], v[:], wVb[:])

            # combine: 4 shifted-identity matmuls into psum
            nrows = TS if t < NT - 1 else NB - TS * (NT - 1)
            ps = psqr.tile([128, 256], f32, tag="psq", name=f"psq{t}")
            nc.tensor.matmul(ps[0:TS, :], lhsT=idm[:, 384:384 + TS],
                             rhs=uw[:, 0:256], start=True, stop=False)
            nc.tensor.matmul(ps[0:TS, :], lhsT=idm[:, 256:256 + TS],
                             rhs=uw[:, 256:512], start=False, stop=False)
            nc.tensor.matmul(ps[0:TS, :], lhsT=idm[:, 128:128 + TS],
                             rhs=vw[:, 0:256], start=False, stop=False)
            nc.tensor.matmul(ps[0:TS, :], lhsT=idm[:, 0:TS],
                             rhs=vw[:, 256:512], start=False, stop=True)
            evict(ps, t, 0, nrows)

            # ch1 (imag): rank-8 taps matmul
            ps1 = psqr.tile([128, 256], f32, tag="psq", name=f"ps1_{t}")
            nc.tensor.matmul(ps1[0:TS, :], lhsT=taps[:, TS * t:TS * t + TS],
                             rhs=tapw[:], start=True, stop=True)
            evict(ps1, t, 1, nrows)

    nc.compile()
    return nc


def _host_tensors(z: np.ndarray, window: np.ndarray):
    """Window-dependent folds, done once per call (host time is free)."""
    import ml_dtypes

    w = window.astype(np.float64)
    ws4 = w[0:256] + w[256:512] + w[512:768] + w[768:1024]
    n4 = np.where(ws4 >= 1e-6, 1.0 / np.where(ws4 >= 1e-6, ws4, 1.0), 1.0)
    ws3a = ws4 - w[768:1024]   # block 0 misses frame -1   (q=3 term)
    ws3b = ws4 - w[0:256]      # block 8190 misses frame 8192 (q=0 term)
    n3a = np.where(ws3a >= 1e-6, 1.0 / np.where(ws3a >= 1e-6, ws3a, 1.0), 1.0)
    n3b = np.where(ws3b >= 1e-6, 1.0 / np.where(ws3b >= 1e-6, ws3b, 1.0), 1.0)

    # A rows (cos 0..512, sin 1..511) over n = 0..511, with n4 folded in.
    j = np.arange(512, dtype=np.float64)[None, :]
    kc = np.arange(513, dtype=np.float64)[:, None]
    g = np.full((513, 1), 2.0)
    g[0, 0] = 1.0
    g[512, 0] = 1.0
    C = (g / N_FFT) * np.cos(2.0 * np.pi * kc * j / N_FFT)
    ks = np.arange(1, 512, dtype=np.float64)[:, None]
    S = (-2.0 / N_FFT) * np.sin(2.0 * np.pi * ks * j / N_FFT)
    n4f = np.tile(n4, 2)[None, :]
    C *= n4f
    S *= n4f
    # even/odd packing: We = [cos k even (257); sin k even (255)]
    #                   Wo = [cos k odd (256); sin k odd (256)]
    We = np.concatenate([C[0::2], S[1::2]], 0)          # [512, 512]
    Wo = np.concatenate([C[1::2], S[0::2]], 0)          # [512, 512]
    W = np.concatenate([We, Wo], 0)                     # [1024, 512]
    w8 = np.ascontiguousarray(
        W.reshape(8, 128, 512).transpose(1, 0, 2).reshape(128, 4096)
    ).astype(ml_dtypes.bfloat16)

    wrows = np.stack([w[0:512], w[512:1024]]).astype(ml_dtypes.bfloat16)

    # ch1 tap weights: rows 0-3 -> w[256q+r]*n4[r]/N ; rows 4-7 -> *(-1)^r
    w4 = w.reshape(4, 256)
    sgn = (1.0 - 2.0 * (np.arange(256) % 2))
    tapw = np.empty((8, 256), np.float64)
    tapw[0:4] = w4 * n4[None, :] / N_FFT
    tapw[4:8] = tapw[0:4] * sgn[None, :]
    tapw = tapw.astype(ml_dtypes.bfloat16)

    fx0 = (n3a * np.where(ws4 >= 1e-6, ws4, 1.0)).astype(np.float32)
    fx7 = (n3b * np.where(ws4 >= 1e-6, ws4, 1.0)).astype(np.float32)
    return w8, wrows, tapw, fx0, fx7


def _id_strips():
    """4 stacked [128,128] shift matrices M_s (out[m] = rhs[m+s]), packed
    [M0 | M1 | M2 | M3] along columns."""
    import ml_dtypes
    idm = np.zeros((128, 512), np.float32)
    for s in range(4):
        for m in range(128 - s):
            idm[m + s, 128 * s + m] = 1.0
    return idm.astype(ml_dtypes.bfloat16)


def _inputs_for_cores(z: np.ndarray, window: np.ndarray):
    import ml_dtypes

    w8, wrows, tapw, fx0, fx7 = _host_tensors(z, window)
    _CACHE["fx"] = (fx0, fx7)
    idm = _CACHE.get("idm")
    if idm is None:
        idm = _id_strips()
        _CACHE["idm"] = idm
    ones = np.ones((1, 128), ml_dtypes.bfloat16)

    # zero-padded zi0 / zi512 rows over all frame slots (halo = 3)
    pad = 8
    zi0 = np.zeros(T_FRAMES + FPAD + pad, np.float64)
    zi0[3:3 + T_FRAMES] = z[1, 0, :]
    zi512 = np.zeros(T_FRAMES + FPAD + pad, np.float64)
    zi512[3:3 + T_FRAMES] = z[1, 512, :]

    in_maps = []
    for c in range(N_CORES):
        G = 1024 * c - 1  # global frame index of slot 0
        X = np.zeros((1024, FPAD), np.float32)
        lo, hi = max(0, G), min(T_FRAMES, G + F_SLOTS)
        s0, s1 = lo - G, hi - G
        zr = z[0, :, lo:hi]          # [513, n]
        zi = z[1, 1:512, lo:hi]      # [511, n]
        # even/odd packing to match We/Wo row order
        X[0:257, s0:s1] = zr[0::2]
        X[257:512, s0:s1] = zi[1::2]     # zi_k k even = 2,4,..,510
        X[512:768, s0:s1] = zr[1::2]
        X[768:1024, s0:s1] = zi[0::2]    # zi_k k odd = 1,3,..,511

        # tile-major repack: xt[128t+p, 128k+s] = X[128k+p, 125t+s]
        X8 = X.reshape(8, 128, FPAD)
        xt = np.empty((128 * NT, 1024), np.float32)
        for t in range(NT):
            blk = X8[:, :, TS * t:TS * t + 128]          # [8, 128, 128]
            xt[128 * t:128 * (t + 1)] = (
                blk.transpose(1, 0, 2).reshape(128, 1024))

        # taps[q, b] = zi0[G + b + 3 - q], taps[4+q, b] = zi512[...]
        taps = np.empty((8, FPAD), np.float64)
        for q in range(4):
            base = G + 3 - q + 3
            taps[q] = zi0[base:base + FPAD]
            taps[4 + q] = zi512[base:base + FPAD]

        in_maps.append({
            "xt": xt.astype(ml_dtypes.bfloat16),
            "w8": w8,
            "onesr": ones,
            "wrows": wrows,
            "idmats": idm,
            "taps": taps.astype(ml_dtypes.bfloat16),
            "tapw": tapw,
        })
    return in_maps


def kernel(z: np.ndarray, window: np.ndarray) -> np.ndarray:
    from concourse.bass_utils import run_bass_kernel_spmd

    z = np.asarray(z, dtype=np.float32)
    window = np.asarray(window, dtype=np.float32)

    nc = _CACHE.get("nc")
    if nc is None:
        nc = _build_nc()
        _CACHE["nc"] = nc

    in_maps = _inputs_for_cores(z, window)
    res = run_bass_kernel_spmd(nc, in_maps, list(range(N_CORES)))

    parts = []
    for c in range(N_CORES):
        nb = NB if c < N_CORES - 1 else NB - 1
        o = np.asarray(res.results[c]["out"], dtype=np.float32)  # [2, NB, 256]
        parts.append(o[:, :nb, :].reshape(2, -1))
    out = np.ascontiguousarray(np.concatenate(parts, axis=1))
    # edge-block window-sum fixup (blocks 0 and 8190), host-side
    fx0, fx7 = _CACHE["fx"]
    out[:, 0:256] *= fx0[None, :]
    out[:, -256:] *= fx7[None, :]
    return out


# revision 34
# speedup vs baseline: 1.2209x; 1.0110x over previous
"""Distributed ISTFT kernel for Trainium2 (8 NeuronCores, Bass/Tile).

Math (matches the jax reference):
  z: [2, 513, T] one-sided spectrum (real/imag), T = 8192 frames.
  Hermitian extension + ifft(1024) + window + overlap-add (hop 256) +
  divide by overlapped window sum + trim 512 each side -> [2, 2096896].

Structure (even/odd spectral symmetry, per-tile pipeline):
  * real(ifft) frame x_f[n] = sum_kappa A[kappa, n] X[kappa, f] where A
    packs cos rows (zr bins 0..512) and sin rows (zi bins 1..511).
    Since A[k, n+512] = (-1)^k A[k, n], split rows into even-k / odd-k
    halves and compute only n = 0..511 (half the matmul rows):
        E_f = We^T Xe,  O_f = Wo^T Xo
        x_f[n] = E+O,  x_f[n+512] = E-O
    The 1/win_sum normalization is 256-periodic in n, identical for the
    n and n+512 uses, so it folds into We/Wo columns on the host.  The
    window itself does not (w[n] != w[n+512]); it is applied with the
    +- combine as four bf16 DVE ops per tile (Uw = w[:512]*(E+O),
    Vw = w[512:]*(E-O)).
  * Frames are tiled 128 slots per tile at a 125 stride; output tiles
    are 125 blocks, so an output tile's whole frame range (125+3) lives
    in ONE frame tile.  Overlap-add for block b, sample r:
      O[b,r] = Uw[b+3, r] + Uw[b+2, 256+r] + Vw[b+1, r] + Vw[b, 256+r]
    The +1/+2/+3 partition shifts are illegal for engine APs, so they
    run as 4 shifted-identity matmuls accumulating in PSUM.
  * X is repacked tile-major on the host (one DMA per frame tile) so
    each tile's matmuls are independent of the rest of the stream: the
    kernel is a 9-deep per-tile pipeline and the PE never idles.
  * imag(ifft) is rank-2 -> one K=8 matmul per output tile with
    host-preshifted tap rows.
  * All matmul operands bf16; outputs written bf16, host upcasts.
  * Frame axis sharded 1024 output blocks/core with a 3-frame halo; no
    cross-core communication.  Window-sum edge fixup (global blocks 0
    and 8190, 512 samples) is applied host-side after the gather.
"""

import numpy as np

N_FFT = 1024
HOP = 256
T_FRAMES = 8192
N_CORES = 8
F_SLOTS = 1027   # valid frame slots per core (1024 owned blocks + 3 halo)
FPAD = 1128      # padded slots: tile 8 covers 1000..1127
NT = 9           # frame/output tiles per core
TS = 125         # output-tile stride (blocks per tile)
NB = 1024        # output blocks computed per core (core 7 uses 1023)

_CACHE = {}


def _build_nc():
    from contextlib import ExitStack

    import concourse.tile as tile
    from concourse import bacc, mybir

    f32 = mybir.dt.float32
    bf16 = mybir.dt.bfloat16

    nc = bacc.Bacc("TRN2", target_bir_lowering=False, debug=False,
                   num_devices=N_CORES)

    # tile-major X: rows 128*t..128*t+127 hold frame tile t as
    # [kappa-in-chunk, 128*k + slot]
    x_d = nc.dram_tensor("xt", [128 * NT, 1024], bf16, kind="ExternalInput")
    # chunk-major W: w8[p, 512*k + j] = W[128*k + p, j]
    w_d = nc.dram_tensor("w8", [128, 4096], bf16, kind="ExternalInput")
    ones_d = nc.dram_tensor("onesr", [1, 128], bf16, kind="ExternalInput")
    wrow_d = nc.dram_tensor("wrows", [2, 512], bf16, kind="ExternalInput")
    id_d = nc.dram_tensor("idmats", [128, 512], bf16, kind="ExternalInput")
    t_d = nc.dram_tensor("taps", [8, FPAD], bf16, kind="ExternalInput")
    tw_d = nc.dram_tensor("tapw", [8, 256], bf16, kind="ExternalInput")
    o_d = nc.dram_tensor("out", [2, NB, 256], bf16, kind="ExternalOutput")

    with tile.TileContext(nc) as tc, ExitStack() as ctx:
        big = ctx.enter_context(tc.tile_pool(name="big", bufs=1))
        xtp = ctx.enter_context(tc.tile_pool(name="xtp", bufs=3))
        sml = ctx.enter_context(tc.tile_pool(name="sml", bufs=1))
        eop = ctx.enter_context(tc.tile_pool(name="eop", bufs=3))
        uvp = ctx.enter_context(tc.tile_pool(name="uvp", bufs=3))
        osb = ctx.enter_context(tc.tile_pool(name="osb", bufs=6))

        SY, SC, GP = nc.sync, nc.scalar, nc.gpsimd

        # ---- tiny setup loads + window broadcast tiles (transient psum)
        onesr = sml.tile([1, 128], bf16, tag="onesr")
        SY.dma_start(out=onesr[:], in_=ones_d.ap())
        wrowU = sml.tile([1, 512], bf16, tag="wrowU")
        SY.dma_start(out=wrowU[:], in_=wrow_d.ap()[0:1, :])
        wrowV = sml.tile([1, 512], bf16, tag="wrowV")
        SC.dma_start(out=wrowV[:], in_=wrow_d.ap()[1:2, :])
        idm = sml.tile([128, 512], bf16, tag="idm")

        wUb = sml.tile([128, 512], bf16, tag="wUb")
        wVb = sml.tile([128, 512], bf16, tag="wVb")
        with tc.tile_pool(name="psw", bufs=1, space="PSUM") as psw:
            ps_wb = psw.tile([128, 512], f32, tag="ps_wb")
            nc.tensor.matmul(ps_wb[:], lhsT=onesr[:], rhs=wrowU[:],
                             start=True, stop=True)
            nc.scalar.copy(wUb[:], ps_wb[:])
            ps_wb2 = psw.tile([128, 512], f32, tag="ps_wb")
            nc.tensor.matmul(ps_wb2[:], lhsT=onesr[:], rhs=wrowV[:],
                             start=True, stop=True)
            nc.scalar.copy(wVb[:], ps_wb2[:])

        mainps = ctx.enter_context(tc.tile_pool(name="mainps", bufs=4,
                                                space="PSUM"))
        psqr = ctx.enter_context(tc.tile_pool(name="psqr", bufs=3,
                                              space="PSUM"))

        # ---- feed: tile-0's needs first, then pipelined X tiles
        ws = big.tile([128, 4096], bf16, tag="w8")
        xt = [xtp.tile([128, 1024], bf16, tag="xt", name=f"xt{t}")
              for t in range(NT)]
        taps = sml.tile([8, FPAD], bf16, tag="taps")
        tapw = sml.tile([8, 256], bf16, tag="tapw")

        def ldw(q, c0, c1):
            q.dma_start(out=ws[:, c0:c1], in_=w_d.ap()[:, c0:c1])

        def ldx(q, t):
            q.dma_start(out=xt[t][:], in_=x_d.ap()[128 * t:128 * (t + 1), :])

        ldw(SY, 0, 1024)        # We chunks 0-1
        ldw(SC, 1024, 2048)     # We chunks 2-3
        ldx(GP, 0)
        ldw(SY, 2048, 3072)     # Wo chunks 4-5
        ldw(SC, 3072, 4096)     # Wo chunks 6-7
        ldx(GP, 1)
        ldx(SY, 2)
        ldx(SC, 3)
        GP.dma_start(out=idm[:], in_=id_d.ap())
        ldx(GP, 4)
        ldx(SY, 5)
        SC.dma_start(out=taps[:], in_=t_d.ap())
        SC.dma_start(out=tapw[:], in_=tw_d.ap())
        ldx(SC, 6)
        ldx(SY, 7)
        ldx(SC, 8)

        oq = [SY, GP, SC]
        ev_state = {"n": 0}

        # edge-block window-sum fixups (global blocks 0 / 8190) are
        # applied host-side after the gather.
        def evict(ps, tt, ch, nrows):
            i = ev_state["n"]
            ev_state["n"] += 1
            o = osb.tile([128, 256], bf16, tag=f"o{ch}", name=f"o{ch}_{tt}")
            if i % 2 == 0:
                nc.vector.tensor_copy(o[0:nrows, :], ps[0:nrows, :])
            else:
                nc.scalar.copy(o[0:nrows, :], ps[0:nrows, :])
            oq[i % 3].dma_start(
                out=o_d.ap()[ch:ch + 1, TS * tt:TS * tt + nrows, :],
                in_=o[0:nrows, :])

        # ---- 9-deep per-tile pipeline; the combine for tile t-1 is
        # emitted after tile t's main matmuls so the in-order PE never
        # waits on the DVE/Pool prep of the tile it just produced.
        uws = [uvp.tile([128, 512], bf16, tag="uw", name=f"uw{t}")
               for t in range(NT)]
        vws = [uvp.tile([128, 512], bf16, tag="vw", name=f"vw{t}")
               for t in range(NT)]

        def main_tile(t):
            x = xt[t]
            psE = mainps.tile([128, 512], f32, tag="mps", name=f"psE{t}")
            for k in range(4):
                nc.tensor.matmul(psE[:], lhsT=x[:, 128 * k:128 * (k + 1)],
                                 rhs=ws[:, 512 * k:512 * (k + 1)],
                                 start=(k == 0), stop=(k == 3))
            eb = eop.tile([128, 512], bf16, tag="eb", name=f"eb{t}")
            if t % 3 == 2:
                nc.vector.tensor_copy(eb[:], psE[:])
            else:
                nc.scalar.copy(eb[:], psE[:])
            psO = mainps.tile([128, 512], f32, tag="mps", name=f"psO{t}")
            for k in range(4, 8):
                nc.tensor.matmul(psO[:], lhsT=x[:, 128 * k:128 * (k + 1)],
                                 rhs=ws[:, 512 * k:512 * (k + 1)],
                                 start=(k == 4), stop=(k == 7))
            ob = eop.tile([128, 512], bf16, tag="ob", name=f"ob{t}")
            if t % 3 == 2:
                nc.vector.tensor_copy(ob[:], psO[:])
            else:
                nc.scalar.copy(ob[:], psO[:])

            # prep: +- on DVE, window multiplies split Pool/DVE
            u = uvp.tile([128, 512], bf16, tag="u", name=f"u{t}")
            nc.vector.tensor_add(u[:], eb[:], ob[:])
            v = uvp.tile([128, 512], bf16, tag="v", name=f"v{t}")
            nc.vector.tensor_sub(v[:], eb[:], ob[:])
            if t % 2 == 0:
                nc.gpsimd.tensor_mul(uws[t][:], u[:], wUb[:])
                nc.vector.tensor_mul(vws[t][:], v[:], wVb[:])
            else:
                nc.vector.tensor_mul(uws[t][:], u[:], wUb[:])
                nc.gpsimd.tensor_mul(vws[t][:], v[:], wVb[:])

        def combine(t):
            nrows = TS if t < NT - 1 else NB - TS * (NT - 1)
            uw, vw = uws[t], vws[t]
            ps = psqr.tile([128, 256], f32, tag="psq", name=f"psq{t}")
            nc.tensor.matmul(ps[0:TS, :], lhsT=idm[:, 384:384 + TS],
                             rhs=uw[:, 0:256], start=True, stop=False)
            nc.tensor.matmul(ps[0:TS, :], lhsT=idm[:, 256:256 + TS],
                             rhs=uw[:, 256:512], start=False, stop=False)
            nc.tensor.matmul(ps[0:TS, :], lhsT=idm[:, 128:128 + TS],
                             rhs=vw[:, 0:256], start=False, stop=False)
            nc.tensor.matmul(ps[0:TS, :], lhsT=idm[:, 0:TS],
                             rhs=vw[:, 256:512], start=False, stop=True)
            evict(ps, t, 0, nrows)
            # ch1 (imag): rank-8 taps matmul
            ps1 = psqr.tile([128, 256], f32, tag="psq", name=f"ps1_{t}")
            nc.tensor.matmul(ps1[0:TS, :], lhsT=taps[:, TS * t:TS * t + TS],
                             rhs=tapw[:], start=True, stop=True)
            evict(ps1, t, 1, nrows)

        for t in range(NT):
            main_tile(t)
            if t >= 1:
                combine(t - 1)
        combine(NT - 1)

    nc.compile()
    return nc


def _host_tensors(z: np.ndarray, window: np.ndarray):
    """Window-dependent folds, done once per call (host time is free)."""
    import ml_dtypes

    w = window.astype(np.float64)
    ws4 = w[0:256] + w[256:512] + w[512:768] + w[768:1024]
    n4 = np.where(ws4 >= 1e-6, 1.0 / np.where(ws4 >= 1e-6, ws4, 1.0), 1.0)
    ws3a = ws4 - w[768:1024]   # block 0 misses frame -1   (q=3 term)
    ws3b = ws4 - w[0:256]      # block 8190 misses frame 8192 (q=0 term)
    n3a = np.where(ws3a >= 1e-6, 1.0 / np.where(ws3a >= 1e-6, ws3a, 1.0), 1.0)
    n3b = np.where(ws3b >= 1e-6, 1.0 / np.where(ws3b >= 1e-6, ws3b, 1.0), 1.0)

    # A rows (cos 0..512, sin 1..511) over n = 0..511, with n4 folded in.
    j = np.arange(512, dtype=np.float64)[None, :]
    kc = np.arange(513, dtype=np.float64)[:, None]
    g = np.full((513, 1), 2.0)
    g[0, 0] = 1.0
    g[512, 0] = 1.0
    C = (g / N_FFT) * np.cos(2.0 * np.pi * kc * j / N_FFT)
    ks = np.arange(1, 512, dtype=np.float64)[:, None]
    S = (-2.0 / N_FFT) * np.sin(2.0 * np.pi * ks * j / N_FFT)
    n4f = np.tile(n4, 2)[None, :]
    C *= n4f
    S *= n4f
    # even/odd packing: We = [cos k even (257); sin k even (255)]
    #                   Wo = [cos k odd (256); sin k odd (256)]
    We = np.concatenate([C[0::2], S[1::2]], 0)          # [512, 512]
    Wo = np.concatenate([C[1::2], S[0::2]], 0)          # [512, 512]
    W = np.concatenate([We, Wo], 0)                     # [1024, 512]
    w8 = np.ascontiguousarray(
        W.reshape(8, 128, 512).transpose(1, 0, 2).reshape(128, 4096)
    ).astype(ml_dtypes.bfloat16)

    wrows = np.stack([w[0:512], w[512:1024]]).astype(ml_dtypes.bfloat16)

    # ch1 tap weights: rows 0-3 -> w[256q+r]*n4[r]/N ; rows 4-7 -> *(-1)^r
    w4 = w.reshape(4, 256)
    sgn = (1.0 - 2.0 * (np.arange(256) % 2))
    tapw = np.empty((8, 256), np.float64)
    tapw[0:4] = w4 * n4[None, :] / N_FFT
    tapw[4:8] = tapw[0:4] * sgn[None, :]
    tapw = tapw.astype(ml_dtypes.bfloat16)

    fx0 = (n3a * np.where(ws4 >= 1e-6, ws4, 1.0)).astype(np.float32)
    fx7 = (n3b * np.where(ws4 >= 1e-6, ws4, 1.0)).astype(np.float32)
    return w8, wrows, tapw, fx0, fx7


def _id_strips():
    """4 stacked [128,128] shift matrices M_s (out[m] = rhs[m+s]), packed
    [M0 | M1 | M2 | M3] along columns."""
    import ml_dtypes
    idm = np.zeros((128, 512), np.float32)
    for s in range(4):
        for m in range(128 - s):
            idm[m + s, 128 * s + m] = 1.0
    return idm.astype(ml_dtypes.bfloat16)


def _inputs_for_cores(z: np.ndarray, window: np.ndarray):
    import ml_dtypes

    w8, wrows, tapw, fx0, fx7 = _host_tensors(z, window)
    _CACHE["fx"] = (fx0, fx7)
    idm = _CACHE.get("idm")
    if idm is None:
        idm = _id_strips()
        _CACHE["idm"] = idm
    ones = np.ones((1, 128), ml_dtypes.bfloat16)

    # zero-padded zi0 / zi512 rows over all frame slots (halo = 3)
    pad = 8
    zi0 = np.zeros(T_FRAMES + FPAD + pad, np.float64)
    zi0[3:3 + T_FRAMES] = z[1, 0, :]
    zi512 = np.zeros(T_FRAMES + FPAD + pad, np.float64)
    zi512[3:3 + T_FRAMES] = z[1, 512, :]

    in_maps = []
    for c in range(N_CORES):
        G = 1024 * c - 1  # global frame index of slot 0
        X = np.zeros((1024, FPAD), np.float32)
        lo, hi = max(0, G), min(T_FRAMES, G + F_SLOTS)
        s0, s1 = lo - G, hi - G
        zr = z[0, :, lo:hi]          # [513, n]
        zi = z[1, 1:512, lo:hi]      # [511, n]
        # even/odd packing to match We/Wo row order
        X[0:257, s0:s1] = zr[0::2]
        X[257:512, s0:s1] = zi[1::2]     # zi_k k even = 2,4,..,510
        X[512:768, s0:s1] = zr[1::2]
        X[768:1024, s0:s1] = zi[0::2]    # zi_k k odd = 1,3,..,511

        # tile-major repack: xt[128t+p, 128k+s] = X[128k+p, 125t+s]
        X8 = X.reshape(8, 128, FPAD)
        xt = np.empty((128 * NT, 1024), np.float32)
        for t in range(NT):
            blk = X8[:, :, TS * t:TS * t + 128]          # [8, 128, 128]
            xt[128 * t:128 * (t + 1)] = (
                blk.transpose(1, 0, 2).reshape(128, 1024))

        # taps[q, b] = zi0[G + b + 3 - q], taps[4+q, b] = zi512[...]
        taps = np.empty((8, FPAD), np.float64)
        for q in range(4):
            base = G + 3 - q + 3
            taps[q] = zi0[base:base + FPAD]
            taps[4 + q] = zi512[base:base + FPAD]

        in_maps.append({
            "xt": xt.astype(ml_dtypes.bfloat16),
            "w8": w8,
            "onesr": ones,
            "wrows": wrows,
            "idmats": idm,
            "taps": taps.astype(ml_dtypes.bfloat16),
            "tapw": tapw,
        })
    return in_maps


def kernel(z: np.ndarray, window: np.ndarray) -> np.ndarray:
    from concourse.bass_utils import run_bass_kernel_spmd

    z = np.asarray(z, dtype=np.float32)
    window = np.asarray(window, dtype=np.float32)

    nc = _CACHE.get("nc")
    if nc is None:
        nc = _build_nc()
        _CACHE["nc"] = nc

    in_maps = _inputs_for_cores(z, window)
    res = run_bass_kernel_spmd(nc, in_maps, list(range(N_CORES)))

    parts = []
    for c in range(N_CORES):
        nb = NB if c < N_CORES - 1 else NB - 1
        o = np.asarray(res.results[c]["out"], dtype=np.float32)  # [2, NB, 256]
        parts.append(o[:, :nb, :].reshape(2, -1))
    out = np.ascontiguousarray(np.concatenate(parts, axis=1))
    # edge-block window-sum fixup (blocks 0 and 8190), host-side
    fx0, fx7 = _CACHE["fx"]
    out[:, 0:256] *= fx0[None, :]
    out[:, -256:] *= fx7[None, :]
    return out


# revision 35
# speedup vs baseline: 1.2553x; 1.0282x over previous
"""Distributed ISTFT kernel for Trainium2 (8 NeuronCores, Bass/Tile).

Math (matches the jax reference):
  z: [2, 513, T] one-sided spectrum (real/imag), T = 8192 frames.
  Hermitian extension + ifft(1024) + window + overlap-add (hop 256) +
  divide by overlapped window sum + trim 512 each side -> [2, 2096896].

Folds used here:
  * real(ifft) = A^T @ X where A [1024(k), 1024(n)] packs the cos rows for
    zr bins 0..512 and sin rows for zi bins 1..511; X packs those z rows.
  * imag(ifft)[n, t] = (zi[0,t] + (-1)^n zi[512,t]) / N  (rank-2) -> one
    K=8 matmul per output tile with host-preshifted tap rows.
  * Output sample m = 256*b + r; block b = sum_{q=0..3} wf_{b-q}[256q+r].
    The window AND the reciprocal window-sum are folded into A on the
    HOST (Awn[kappa, n] = A[kappa, n] * w[n] / ws4[n mod 256]), so the
    overlap-add, windowing and normalization all ride inside the matmul
    and psum eviction is a plain copy.
  * All matmul operands are bf16 (PE streams 1 row/cycle either way, but
    DMA bytes and LDWEIGHTS time halve; rel-err ~3e-3 << 2e-2 budget).
  * Frame axis is sharded 1024 output blocks/core with a 3-frame input
    halo, so no cross-core communication is needed.  The two blocks
    whose window-sum misses a frame (global 0 and 8190) get a
    data-driven single-row fixup (factor is 1.0 on non-edge cores).
"""

import numpy as np

N_FFT = 1024
HOP = 256
T_FRAMES = 8192
N_CORES = 8
F_SLOTS = 1027  # frame slots per core: 1024 owned blocks need slots b..b+3
NB = 1024       # output blocks computed per core (core 7 uses 1023)

_CACHE = {}


def _amat() -> np.ndarray:
    """A [1024(kappa), 1024(n)]: ifft cos/sin weights, f64 precision."""
    n = np.arange(N_FFT, dtype=np.float64)[None, :]
    k = np.arange(513, dtype=np.float64)[:, None]
    g = np.full((513, 1), 2.0)
    g[0, 0] = 1.0
    g[512, 0] = 1.0
    C = (g / N_FFT) * np.cos(2.0 * np.pi * k * n / N_FFT)
    k2 = np.arange(1, 512, dtype=np.float64)[:, None]
    S = (-2.0 / N_FFT) * np.sin(2.0 * np.pi * k2 * n / N_FFT)
    return np.concatenate([C, S], 0)  # [1024, 1024] f64


def _build_nc():
    from contextlib import ExitStack

    import concourse.tile as tile
    from concourse import bacc, mybir

    f32 = mybir.dt.float32
    bf16 = mybir.dt.bfloat16

    nc = bacc.Bacc("TRN2", target_bir_lowering=False, debug=False,
                   num_devices=N_CORES)

    x_d = nc.dram_tensor("x", [1024, F_SLOTS], bf16, kind="ExternalInput")
    a_d = nc.dram_tensor("awn", [1024, 1024], bf16, kind="ExternalInput")
    t_d = nc.dram_tensor("taps", [8, NB], bf16, kind="ExternalInput")
    tw_d = nc.dram_tensor("tapw", [8, 256], bf16, kind="ExternalInput")
    o_d = nc.dram_tensor("out", [2, NB, 256], bf16, kind="ExternalOutput")

    with tile.TileContext(nc) as tc, ExitStack() as ctx:
        big = ctx.enter_context(tc.tile_pool(name="big", bufs=1))
        sml = ctx.enter_context(tc.tile_pool(name="sml", bufs=1))
        ps0p = ctx.enter_context(tc.tile_pool(name="ps0p", bufs=6, space="PSUM"))
        ps1p = ctx.enter_context(tc.tile_pool(name="ps1p", bufs=2, space="PSUM"))
        osb = ctx.enter_context(tc.tile_pool(name="osb", bufs=8))

        # ---- big input chunks interleaved across the 3 DMA queues in
        # k-order so pair (xs[k], awn[k]) lands just ahead of the PE's
        # k-step.  k=0 is halved for a fast pipeline start.
        xs = [big.tile([128, F_SLOTS], bf16, tag=f"xs{k}", name=f"xs{k}")
              for k in range(8)]
        aw = [big.tile([128, 1024], bf16, tag=f"aw{k}", name=f"aw{k}")
              for k in range(8)]

        def ld_x(q, k, c0, c1):
            q.dma_start(out=xs[k][:, c0:c1], in_=x_d.ap()[128 * k:128 * (k + 1), c0:c1])

        def ld_a(q, k, c0, c1):
            q.dma_start(out=aw[k][:, c0:c1], in_=a_d.ap()[128 * k:128 * (k + 1), c0:c1])

        SY, SC, GP = nc.sync, nc.scalar, nc.gpsimd
        sched = [
            (ld_x, SY, 0, 0, 515), (ld_a, SC, 0, 0, 512), (ld_x, GP, 0, 515, 1027),
            (ld_a, GP, 0, 512, 1024),
            (ld_x, SY, 1, 0, 1027), (ld_a, SC, 1, 0, 1024),
            (ld_x, GP, 2, 0, 1027), (ld_a, SY, 2, 0, 1024),
            (ld_x, SC, 3, 0, 1027), (ld_a, GP, 3, 0, 1024),
            (ld_x, SY, 4, 0, 1027), (ld_a, SC, 4, 0, 1024),
            (ld_x, GP, 5, 0, 1027), (ld_a, SY, 5, 0, 1024),
            (ld_x, SC, 6, 0, 1027), (ld_a, GP, 6, 0, 1024),
            (ld_x, SY, 7, 0, 1027), (ld_a, SC, 7, 0, 1024),
        ]
        for fn, q, k, c0, c1 in sched:
            fn(q, k, c0, c1)

        # ---- tiny setup loads (needed only mid-kernel)
        taps = sml.tile([8, NB], bf16, tag="taps")
        nc.gpsimd.dma_start(out=taps[:], in_=t_d.ap())
        tapw = sml.tile([8, 256], bf16, tag="tapw")
        nc.gpsimd.dma_start(out=tapw[:], in_=tw_d.ap())

        oq = [nc.sync, nc.gpsimd, nc.scalar]
        ev_state = {"n": 0}

        # edge-block window-sum fixups (global blocks 0 / 8190) are applied
        # host-side after the gather -- they touch only 512 samples.
        def evict(ps, tt, ch):
            i = ev_state["n"]
            ev_state["n"] += 1
            o = osb.tile([128, 256], bf16, tag=f"o{ch}", name=f"o{ch}_{tt}")
            if i % 2 == 0:
                nc.vector.tensor_copy(o[:], ps[:])
            else:
                nc.scalar.copy(o[:], ps[:])
            oq[i % 3].dma_start(
                out=o_d.ap()[ch:ch + 1, tt * 128:(tt + 1) * 128, :], in_=o[:])

        def ch1_group(tt):
            ps1 = ps1p.tile([128, 256], f32, tag="ps1", name=f"ps1_{tt}")
            nc.tensor.matmul(ps1[:], lhsT=taps[:, tt * 128:tt * 128 + 128],
                             rhs=tapw[:], start=True, stop=True)
            evict(ps1, tt, 1)

        def mm(ps, tt, k, q):
            off = tt * 128 + 3 - q
            nc.tensor.matmul(ps[:], lhsT=xs[k][:, off:off + 128],
                             rhs=aw[k][:, 256 * q:256 * (q + 1)],
                             start=(k == 0 and q == 0),
                             stop=(k == 7 and q == 3))

        # ---- channel 0.  Sweep A: tiles 0-4 k-outer (matches the input
        # stream's pair cadence).  Then tiles 5-7 k-inner with the sweep-A
        # evictions and ch1 groups spread through the slack.
        pss = {tt: ps0p.tile([128, 256], f32, tag="ps0", name=f"ps0_{tt}")
               for tt in range(6)}
        for k in range(8):
            for tt in range(6):
                for q in range(4):
                    mm(pss[tt], tt, k, q)
        for tt in range(6):
            evict(pss[tt], tt, 0)

        ch1_sched = {6: [0, 1, 2, 4], 7: [5, 6, 7]}
        for tt in (6, 7):
            ps = ps0p.tile([128, 256], f32, tag="ps0", name=f"ps0_{tt}")
            groups = list(ch1_sched[tt])
            for k in range(8):
                for q in range(4):
                    mm(ps, tt, k, q)
                if k in (1, 3, 5, 7) and groups:
                    ch1_group(groups.pop(0))
            evict(ps, tt, 0)
        ch1_group(3)

    nc.compile()
    return nc


def _host_tensors(z: np.ndarray, window: np.ndarray):
    """Window-dependent folds, done once per call (host time is free)."""
    import ml_dtypes

    amat = _CACHE.get("amat")
    if amat is None:
        amat = _amat()
        _CACHE["amat"] = amat

    w = window.astype(np.float64)
    ws4 = w[0:256] + w[256:512] + w[512:768] + w[768:1024]
    n4 = np.where(ws4 >= 1e-6, 1.0 / np.where(ws4 >= 1e-6, ws4, 1.0), 1.0)
    ws3a = ws4 - w[768:1024]   # block 0 misses frame -1   (q=3 term)
    ws3b = ws4 - w[0:256]      # block 8190 misses frame 8192 (q=0 term)
    n3a = np.where(ws3a >= 1e-6, 1.0 / np.where(ws3a >= 1e-6, ws3a, 1.0), 1.0)
    n3b = np.where(ws3b >= 1e-6, 1.0 / np.where(ws3b >= 1e-6, ws3b, 1.0), 1.0)

    # Awn[kappa, n] = A[kappa, n] * w[n] / ws4[n mod 256]
    colf = w * np.tile(n4, 4)
    awn = (amat * colf[None, :]).astype(ml_dtypes.bfloat16)

    # ch1 tap weights: rows 0-3 -> w[256q+r]*n4[r]/N ; rows 4-7 -> *(-1)^r
    w4 = w.reshape(4, 256)
    sgn = (1.0 - 2.0 * (np.arange(256) % 2))
    tapw = np.empty((8, 256), np.float64)
    tapw[0:4] = w4 * n4[None, :] / N_FFT
    tapw[4:8] = tapw[0:4] * sgn[None, :]
    tapw = tapw.astype(ml_dtypes.bfloat16)

    fx0 = (n3a * np.where(ws4 >= 1e-6, ws4, 1.0)).astype(np.float32)
    fx7 = (n3b * np.where(ws4 >= 1e-6, ws4, 1.0)).astype(np.float32)
    return awn, tapw, fx0, fx7


def _inputs_for_cores(z: np.ndarray, window: np.ndarray):
    import ml_dtypes

    awn, tapw, fx0, fx7 = _host_tensors(z, window)
    _CACHE["fx"] = (fx0, fx7)

    # zero-padded zi0 / zi512 rows over all frame slots (halo = 3)
    zi0 = np.zeros(T_FRAMES + 6, np.float64)
    zi0[3:3 + T_FRAMES] = z[1, 0, :]
    zi512 = np.zeros(T_FRAMES + 6, np.float64)
    zi512[3:3 + T_FRAMES] = z[1, 512, :]

    in_maps = []
    for c in range(N_CORES):
        G = 1024 * c - 1  # global frame index of slot 0
        X = np.zeros((1024, F_SLOTS), np.float32)
        lo, hi = max(0, G), min(T_FRAMES, G + F_SLOTS)
        s0, s1 = lo - G, hi - G
        X[0:513, s0:s1] = z[0, :, lo:hi]
        X[513:1024, s0:s1] = z[1, 1:512, lo:hi]

        # taps[q, b] = zi0[G + b + 3 - q], taps[4+q, b] = zi512[...]
        taps = np.empty((8, NB), np.float64)
        for q in range(4):
            base = G + 3 - q + 3  # +3 for zi0's zero pad offset
            taps[q] = zi0[base:base + NB]
            taps[4 + q] = zi512[base:base + NB]

        in_maps.append({
            "x": X.astype(ml_dtypes.bfloat16),
            "awn": awn,
            "taps": taps.astype(ml_dtypes.bfloat16),
            "tapw": tapw,
        })
    return in_maps


def kernel(z: np.ndarray, window: np.ndarray) -> np.ndarray:
    from concourse.bass_utils import run_bass_kernel_spmd

    z = np.asarray(z, dtype=np.float32)
    window = np.asarray(window, dtype=np.float32)

    nc = _CACHE.get("nc")
    if nc is None:
        nc = _build_nc()
        _CACHE["nc"] = nc

    in_maps = _inputs_for_cores(z, window)
    res = run_bass_kernel_spmd(nc, in_maps, list(range(N_CORES)))

    parts = []
    for c in range(N_CORES):
        nb = NB if c < N_CORES - 1 else NB - 1
        o = np.asarray(res.results[c]["out"], dtype=np.float32)  # [2, NB, 256]
        parts.append(o[:, :nb, :].reshape(2, -1))
    out = np.ascontiguousarray(np.concatenate(parts, axis=1))
    # edge-block window-sum fixup (blocks 0 and 8190), host-side
    fx0, fx7 = _CACHE["fx"]
    out[:, 0:256] *= fx0[None, :]
    out[:, -256:] *= fx7[None, :]
    return out
